# revision 2
# baseline (speedup 1.0000x reference)
"""Trainium2 Bass kernel for a single-step LSTM cell — fp8 DoubleRow, v11.

Change vs baseline: each matmul's stationary operand is a dedicated
whole [128, 2, 128] SBUF tile (plain DoubleRow layout), one per
(K-pair, output block). At the 512-wide free dim, a matmul whose lhsT
is ANY nonzero-offset slice of a larger tile runs ~237 ns/MM on HW
(both the baseline's big SwInterleave rearrange slices and clean
[128, nblk, 2, 128] block slices); a whole-tile lhsT reaches the
~182 ns/MM DoubleRow stream roofline (probe.py).

Everything else follows the baseline: tensor-parallel over hidden
(8 cores x 256 rows), fp8e4 DoubleRow matmuls of 32*(W - mu), exact
rank-1 correction r = 32*mu*colsum(A) broadcast across partitions and
DVE-added into each PSUM bank, gate activation with scale 1/32 + fp32
bias, in-place DVE epilogue, fp32 outputs.
"""

import os

import numpy as np

os.environ.setdefault("JAX_COMPILATION_CACHE_DIR", "/tmp/jax_cache")
os.environ.setdefault("JAX_PLATFORMS", "axon,cpu")

N_CORES = 8
IN_SIZE = 2048
HIDDEN = 2048
BATCH = 4096
P = 128  # SBUF/PSUM partitions
NB = 512  # batch tile (matmul free dim; one PSUM bank of fp32).
# nb=256 chains run at ~52 ns/MM vs ~182 at 512, but need 2x the DMA
# instructions (~625 ns issue overhead each), which dominates; 512
# keeps the DMA count at the baseline's proven level.
G = 4  # gates: i, f, g, o
MU = 0.203125  # weight mean, exactly representable in e4m3
WSCALE = 32.0  # delta pre-scale; PSUM holds 32*(pre-act - bias)


def build_lstm_nc(
    in_size, hid_size, shard, batch, nb=NB, reps=1, loop_reps=0,
    mm_only=False, no_corr=False,
):
    """Build + compile the Bass program (identical NEFF for every core)."""
    import concourse.bass as bass
    import concourse.tile as tile
    from concourse import bacc, mybir
    from concourse._compat import get_trn_type

    f32 = mybir.dt.float32
    f8 = mybir.dt.float8e4
    DR = mybir.MatmulPerfMode.DoubleRow
    AF = mybir.ActivationFunctionType
    gate_funcs = [AF.Sigmoid, AF.Sigmoid, AF.Tanh, AF.Sigmoid]

    k_total = in_size + hid_size
    assert shard % P == 0 and k_total % (2 * P) == 0
    assert batch % nb == 0
    m_tiles = shard // P
    npairs = k_total // (2 * P)
    nn = batch // nb
    nblk = G * m_tiles

    nc = bacc.Bacc(get_trn_type() or "TRN2", target_bir_lowering=False, debug=False)

    # Pair-packed combined rhs (x rows 0..in_size, h.T rows after):
    # a_d[p + P*j, 2*nb*n + nb*i + c] = A[2*P*j + P*i + p, nb*n + c]
    a_d = nc.dram_tensor("a", [npairs * P, 2 * batch], f8, kind="ExternalInput")
    # Exact fp32 correction row (32*mu*colsum(A)).
    r_d = nc.dram_tensor("r", [1, batch], f32, kind="ExternalInput")
    # Weights, DoubleRow block layout:
    # w_d[p + P*j, blk*2*P + i*P + q] = 32*(W^T - mu)[2*P*j + P*i + p,
    #                                               blk*P + q]
    w_d = nc.dram_tensor("w", [npairs * P, nblk * 2 * P], f8, kind="ExternalInput")
    ct_d = nc.dram_tensor("ct", [shard, batch], f32, kind="ExternalInput")
    b_d = nc.dram_tensor("bias", [P, nblk], f32, kind="ExternalInput")
    ho_d = nc.dram_tensor("h_out", [shard, batch], f32, kind="ExternalOutput")
    co_d = nc.dram_tensor("c_out", [shard, batch], f32, kind="ExternalOutput")

    with tile.TileContext(nc) as tc:
        with (
            tc.tile_pool(name="wpool", bufs=1) as wpool,
            tc.tile_pool(name="xpool", bufs=40) as xpool,
            tc.tile_pool(name="rpool", bufs=1) as rpool,
            tc.tile_pool(name="cpool", bufs=4) as cpool,
            tc.tile_pool(name="gpool", bufs=4) as gpool,
            tc.tile_pool(name="bpool", bufs=1) as bpool,
            tc.tile_pool(name="psum", bufs=1, space=bass.MemorySpace.PSUM) as pspool,
        ):
            # Resident weights: one [128, nblk, 2, 128] tile per K-pair.
            # Preload on the gpsimd (SWDGE) queue so the rhs stream on the
            # sync HWDGE ring isn't stuck behind the weights at start.
            # One dedicated [128, 2, 128] tile per (K-pair, block): at
            # nb=512 a whole-tile lhsT runs ~182 ns/MM vs ~237 for any
            # nonzero-offset slice of a larger tile (probe.py).
            w_sb = []
            for j in range(npairs):
                row = []
                for b in range(nblk):
                    wt = wpool.tile(
                        [P, 2, P], f8, tag=f"w{j}_{b}", name=f"w{j}_{b}"
                    )
                    nc.gpsimd.dma_start(
                        out=wt[:],
                        in_=w_d[j * P : (j + 1) * P, b * 2 * P : (b + 1) * 2 * P],
                    )
                    row.append(wt)
                w_sb.append(row)
            bias_sb = bpool.tile([P, nblk], f32, name="bias_sb")
            nc.gpsimd.dma_start(out=bias_sb[:], in_=b_d[:])
            mm_rhs = None
            if mm_only:
                mm_rhs = xpool.tile([P, 2, nb], f8, tag="mmrhs", name="mm_rhs")
                nc.sync.dma_start(out=mm_rhs[:], in_=a_d[0:P, 0 : 2 * nb])

            def emit_body():
              for rep in range(reps):
                if not no_corr:
                    # Exact f32 correction, PE-free: broadcast the r row
                    # across partitions once, then DVE-add per bank.
                    r_sb = rpool.tile([1, batch], f32, tag="r", name=f"r_{rep}")
                    nc.sync.dma_start(out=r_sb[:], in_=r_d[:])
                    corr_bc = rpool.tile(
                        [P, batch], f32, tag="corr", name=f"corr_{rep}"
                    )
                    nc.gpsimd.partition_broadcast(corr_bc[:], r_sb[:])
                for n in range(nn):
                    ncol = slice(n * nb, (n + 1) * nb)
                    # One PSUM bank per (gate, m): 4 * m_tiles <= 8 banks.
                    ps = [
                        [
                            pspool.tile(
                                [P, nb], f32, tag=f"ps{g}_{m}",
                                name=f"ps{g}_{m}_{n}_{rep}",
                            )
                            for m in range(m_tiles)
                        ]
                        for g in range(G)
                    ]
                    # Stream all rhs tiles for this n-tile first; chains
                    # read them from SBUF.
                    rhs_aps = []
                    for j in range(npairs):
                        if mm_only:
                            rhs_aps.append(mm_rhs[:, :, :])
                        else:
                            rhs_t = xpool.tile(
                                [P, 2, nb], f8, tag="rhs", name=f"rhs{n}_{j}"
                            )
                            nc.sync.dma_start(
                                out=rhs_t[:],
                                in_=a_d[j * P : (j + 1) * P,
                                        n * 2 * nb : (n + 1) * 2 * nb],
                            )
                            rhs_aps.append(rhs_t[:, :, :])
                    # Per-bank sequential chains: bank (g, m) finishes
                    # after its 16 matmuls and drains through DVE/ACT
                    # while the PE continues the other banks.
                    for m in range(m_tiles):
                        ct_t = None
                        if not mm_only:
                            mrow = slice(m * P, (m + 1) * P)
                            ct_t = cpool.tile([P, nb], f32, tag="ct", name=f"ct{n}_{m}")
                            nc.sync.dma_start(out=ct_t[:], in_=ct_d[mrow, ncol])
                        gt = []
                        for g in range(G):
                            blk = g * m_tiles + m
                            for j in range(npairs):
                                nc.tensor.matmul(
                                    ps[g][m][:],
                                    w_sb[j][blk][:],
                                    rhs_aps[j],
                                    start=j == 0,
                                    stop=j == npairs - 1,
                                    perf_mode=DR,
                                )
                            if mm_only:
                                continue
                            if not no_corr:
                                nc.vector.tensor_add(
                                    ps[g][m][:], ps[g][m][:], corr_bc[:, ncol]
                                )
                            gsb = gpool.tile(
                                [P, nb], f32, tag=f"g{g}", name=f"g{g}_{n}_{m}"
                            )
                            nc.scalar.activation(
                                gsb[:],
                                ps[g][m][:],
                                gate_funcs[g],
                                bias=bias_sb[:, blk : blk + 1],
                                scale=1.0 / WSCALE,
                            )
                            gt.append(gsb)
                        if mm_only:
                            continue
                        i_t, f_t, g_t, o_t = gt
                        # In-place epilogue: f <- f*c; i <- i*g; f <- f+i (= c');
                        # g <- tanh(c'); o <- o*g (= h').
                        nc.vector.tensor_mul(f_t[:], f_t[:], ct_t[:])
                        nc.vector.tensor_mul(i_t[:], i_t[:], g_t[:])
                        nc.vector.tensor_add(f_t[:], f_t[:], i_t[:])
                        nc.scalar.activation(g_t[:], f_t[:], AF.Tanh)
                        nc.vector.tensor_mul(o_t[:], o_t[:], g_t[:])
                        nc.sync.dma_start(out=co_d[mrow, ncol], in_=f_t[:])
                        nc.sync.dma_start(out=ho_d[mrow, ncol], in_=o_t[:])
                    del ps

            if loop_reps > 0:
                # Timing-only path. Hint the back-edge to avoid a ~4us
                # I$-miss fetch per iteration distorting the estimate.
                ET = mybir.EngineType
                hints = (
                    (ET.PE, ET.Pool)
                    if mm_only
                    else (ET.PE, ET.SP, ET.Activation, ET.DVE, ET.Pool)
                )
                with tc.For_i(0, loop_reps, 1, hint_engines=hints):
                    emit_body()
            else:
                emit_body()

    nc.compile()
    return nc


_NC_CACHE = {}


def _get_nc(key, *args):
    if key not in _NC_CACHE:
        _NC_CACHE[key] = build_lstm_nc(*args)
    return _NC_CACHE[key]


def prepare_inputs(
    inputs, h, c,
    w_ii, w_if, w_ig, w_io,
    w_hi, w_hf, w_hg, w_ho,
    b_ii, b_hi, b_if, b_hf, b_ig, b_hg, b_io, b_ho,
    n_cores=N_CORES,
):
    """Host-side prep: per-core input maps for the SPMD kernel."""
    import ml_dtypes

    e4 = ml_dtypes.float8_e4m3

    in_size, batch = inputs.shape
    hid = h.shape[1]
    shard = hid // n_cores
    m_tiles = shard // P
    k_total = in_size + hid
    npairs = k_total // (2 * P)
    nn = batch // NB
    nblk = G * m_tiles

    x = np.asarray(inputs, dtype=np.float32)
    ht = np.asarray(h).T.astype(np.float32)
    A = np.concatenate([x, ht], axis=0)  # [k_total, batch]
    aq = np.clip(A, -240.0, 240.0).astype(e4)
    # a_pk[p + P*j, 2*NB*n + NB*i + c] = aq[2*P*j + P*i + p, NB*n + c]
    a_pk = np.ascontiguousarray(
        aq.reshape(npairs, 2, P, nn, NB).transpose(0, 2, 3, 1, 4).reshape(
            npairs * P, 2 * batch
        )
    )
    # Exact correction row: r = 32*mu*colsum(A), added by DVE pre-activation.
    q = (MU * A.sum(axis=0, dtype=np.float64)).astype(np.float32)
    r32 = np.ascontiguousarray((WSCALE * q).reshape(1, batch))
    ct = np.ascontiguousarray(np.asarray(c).T, dtype=np.float32)

    w_in = [w_ii, w_if, w_ig, w_io]
    w_hid = [w_hi, w_hf, w_hg, w_ho]
    biases = [b_ii + b_hi, b_if + b_hf, b_ig + b_hg, b_io + b_ho]

    # Combined per-gate lhsT [k_total, hid]: input rows then hidden rows.
    wT = [
        np.concatenate(
            [np.asarray(wi).T.astype(np.float32), np.asarray(wh).T.astype(np.float32)],
            axis=0,
        )
        for wi, wh in zip(w_in, w_hid)
    ]

    in_maps = []
    for s in range(n_cores):
        rows = slice(s * shard, (s + 1) * shard)
        w_s = np.concatenate([w[:, rows] for w in wT], axis=1)  # [k_total, G*shard]
        w_q = np.clip(WSCALE * (w_s - MU), -240.0, 240.0).astype(e4)
        # w_pk[p + P*j, blk*2*P + i*P + q] = w_q[2*P*j + P*i + p, blk*P + q]
        w_pk = np.ascontiguousarray(
            w_q.reshape(npairs, 2, P, nblk, P)
            .transpose(0, 2, 3, 1, 4)
            .reshape(npairs * P, nblk * 2 * P)
        )
        # bias_sb[p, g*m_tiles + m] = bias_g[s*shard + m*128 + p]
        b_cols = []
        for g in range(G):
            bg = np.asarray(biases[g], dtype=np.float32).reshape(-1)[rows]
            for m in range(m_tiles):
                b_cols.append(bg[m * P : (m + 1) * P])
        bias_s = np.ascontiguousarray(np.stack(b_cols, axis=1), dtype=np.float32)
        in_maps.append(
            {
                "a": a_pk,
                "r": r32,
                "w": w_pk,
                "ct": np.ascontiguousarray(ct[rows, :]),
                "bias": bias_s,
            }
        )
    return in_maps


def run_spmd(nc, in_maps, **kwargs):
    from concourse.bass_utils import run_bass_kernel_spmd

    return run_bass_kernel_spmd(nc, in_maps, core_ids=list(range(len(in_maps))), **kwargs)


def assemble_outputs(results):
    ht_next = np.concatenate([r["h_out"] for r in results], axis=0)
    ct_next = np.concatenate([r["c_out"] for r in results], axis=0)
    return ht_next.T, ct_next.T


def kernel(**inputs):
    in_maps = prepare_inputs(**{k: np.asarray(v) for k, v in inputs.items()})
    in_size, batch = inputs["inputs"].shape
    hid = inputs["h"].shape[1]
    shard = hid // N_CORES
    nc = _get_nc((in_size, hid, shard, batch), in_size, hid, shard, batch)
    res = run_spmd(nc, in_maps)
    return assemble_outputs(res.results)
